# revision 10
# baseline (speedup 1.0000x reference)
"""GCN binding-affinity model on 8 Trainium2 NeuronCores.

Self-contained: builds a Bass/Tile SPMD program, shards the graph across 8
cores (nodes dst-partitioned; per-core edges packed into fixed 128-edge tiles
per 128-node window, split by src half for int16 dma_gather indices), runs via
bass_utils.run_bass_kernel_spmd, and returns the full [256, 1] output.

Math (equivalent to the reference):
  per layer: agg = dinv * ((A + I) @ (dinv * t)) with t the passed features,
  where dinv = rsqrt(indegree + 1).  Self-loops are explicit edges.
  L1 passes t = x (scalar; table stores y = dinv*x replicated to 64 lanes),
  then h1 = relu(dinv*S1*W1 + b1) via K=1 outer product.
  L2 passes t = h1 (64-d), then h2 = relu((S2*dinv) @ W2 + b2).
  L3 passes t = h2 @ W3 (64-d), then h3 = relu(S3*dinv + b3).
  pool: one-hot matmul by graph id + AllReduce + mean + 2-layer MLP.

Gather: per group of GSZ windows, one multi-packet dma_gather per src half
into disjoint chunk ranges of one SBUF tile, then a 128-idx single-packet
fence gather on the same SWDGE queue (per-engine FIFO => fence completion
implies all prior descriptors landed); every consumer matmul gets an explicit
dep on the fence.
"""

import os
import sys
from contextlib import ExitStack

import numpy as np

for _p in ("/opt/trn_rl_repo",):
    if _p not in sys.path and os.path.isdir(_p):
        sys.path.insert(0, _p)

import concourse.bass as bass
import concourse.mybir as mybir
import concourse.tile as tile
from concourse import bacc
from concourse import bass_utils
from concourse.masks import make_identity
from concourse.tile_rust import add_dep_helper

F32 = mybir.dt.float32
I16 = mybir.dt.int16
AF = mybir.ActivationFunctionType
OP = mybir.AluOpType
GSZ = 4  # windows per gather group


class Cfg:
    def __init__(self, n_nodes=50000, n_edges=600000, n_graphs=256,
                 n_cores=8, nw=49, nt_a=11, nt_b=7, half=32768):
        self.N = n_nodes
        self.E = n_edges
        self.G = n_graphs
        self.C = n_cores
        self.NW = nw               # 128-node windows per core
        self.NT_A = nt_a           # tiles/window for src < half
        self.NT_B = nt_b           # tiles/window for src >= half
        self.NT = nt_a + nt_b
        self.HALF = half
        self.NWS = nw * 128
        self.PAD_N = self.C * self.NWS
        assert self.PAD_N >= n_nodes
        assert self.PAD_N - half < 32768 and half <= 32768
        self.GP = 256
        assert n_graphs <= self.GP
        # gather groups: [start_window, n_windows]
        self.groups = [(s, min(GSZ, nw - s)) for s in range(0, nw, GSZ)]

    def key(self):
        return (self.N, self.E, self.G, self.C, self.NW, self.NT_A,
                self.NT_B, self.HALF)


def chunk_of(cfg, wl, t, gn):
    """gb chunk index for window-in-group wl, window-tile t, group size gn."""
    if t < cfg.NT_A:
        return wl * cfg.NT_A + t
    return gn * cfg.NT_A + wl * cfg.NT_B + (t - cfg.NT_A)


# ----------------------------------------------------------------------------
# Host-side sharding / packing
# ----------------------------------------------------------------------------

def compute_caps(n, edge_index, cfg_proto):
    """Data-driven NT_A/NT_B (max tiles needed per window + margin)."""
    half = cfg_proto.HALF
    src = np.asarray(edge_index[0], dtype=np.int64)
    dst = np.asarray(edge_index[1], dtype=np.int64)
    self_nodes = np.arange(n, dtype=np.int64)
    a_src = np.concatenate([src, self_nodes])
    a_dst = np.concatenate([dst, self_nodes])
    win = a_dst >> 7  # global 128-node window
    n_win = cfg_proto.C * cfg_proto.NW
    isa = a_src < half
    cnt_a = np.bincount(win[isa], minlength=n_win)
    cnt_b = np.bincount(win[~isa], minlength=n_win)
    nt_a = int(np.ceil(cnt_a.max() / 128)) + 1
    nt_b = max(int(np.ceil(cnt_b.max() / 128)) + 1, 1)
    return nt_a, nt_b


def wrap16(vals):
    """sequence position i -> [i % 16, i // 16], int16."""
    return np.ascontiguousarray(vals.reshape(-1, 16).T.astype(np.int16))


def rep8(block):
    """replicate a [16, X] int16 block to [128, X] (8 Q7 core groups)."""
    return np.tile(block, (8, 1))


def prep_inputs(cfg, x, W1, b1, W2, b2, W3, b3, lin1_w, lin1_b, lin2_w,
                lin2_b, edge_index, batch):
    N, C, NW, NWS = cfg.N, cfg.C, cfg.NW, cfg.NWS
    NT_A, NT_B, NT, HALF = cfg.NT_A, cfg.NT_B, cfg.NT, cfg.HALF

    src = np.asarray(edge_index[0], dtype=np.int64)
    dst = np.asarray(edge_index[1], dtype=np.int64)
    batch = np.asarray(batch, dtype=np.int64)
    x = np.asarray(x, dtype=np.float32).reshape(-1)

    deg = np.bincount(dst, minlength=N).astype(np.float32)
    x_ext = np.zeros(cfg.PAD_N, np.float32); x_ext[:N] = x
    deg_ext = np.zeros(cfg.PAD_N, np.float32); deg_ext[:N] = deg
    batch_ext = np.full(cfg.PAD_N, -1.0, np.float32)
    batch_ext[:N] = batch.astype(np.float32)

    iota = np.broadcast_to(np.arange(cfg.GP, dtype=np.float32),
                           (128, cfg.GP)).copy()
    cnts = np.bincount(batch, minlength=cfg.GP).astype(np.float32)
    cnts2 = np.ascontiguousarray(cnts.reshape(2, 128).T)  # [128, half]

    # append self-loops, assign to cores by dst
    self_nodes = np.arange(N, dtype=np.int64)
    a_src = np.concatenate([src, self_nodes])
    a_dst = np.concatenate([dst, self_nodes])
    core_of = a_dst // NWS
    order0 = np.argsort(core_of, kind="stable")
    a_src, a_dst, core_sorted = a_src[order0], a_dst[order0], core_of[order0]

    # per-group column offsets in the idx arrays (groups may differ in size)
    colsA_of = [gn * NT_A * 128 // 16 for _, gn in cfg.groups]
    colsB_of = [gn * NT_B * 128 // 16 for _, gn in cfg.groups]
    offA = np.concatenate([[0], np.cumsum(colsA_of)]).astype(int)
    offB = np.concatenate([[0], np.cumsum(colsB_of)]).astype(int)
    TOT_A, TOT_B = int(offA[-1]), int(offB[-1])

    in_maps = []
    for c in range(C):
        base = c * NWS
        lo = np.searchsorted(core_sorted, c, side="left")
        hi = np.searchsorted(core_sorted, c, side="right")
        c_src, c_dst = a_src[lo:hi], a_dst[lo:hi]
        w_of = (c_dst - base) >> 7
        is_a = c_src < HALF
        # order edges by (window, half), then rank within each bucket
        key = w_of * 2 + (~is_a)
        order = np.argsort(key, kind="stable")
        c_src, c_dst, w_of, is_a = (c_src[order], c_dst[order], w_of[order],
                                    is_a[order])
        key = key[order]
        kstart = np.searchsorted(key, np.arange(2 * NW), side="left")
        pos = np.arange(len(key)) - kstart[key]
        cnt = np.searchsorted(key, np.arange(2 * NW), side="right") - kstart
        if cnt[0::2].max(initial=0) > NT_A * 128 or \
           cnt[1::2].max(initial=0) > NT_B * 128:
            raise ValueError("window half overflow; increase caps")

        # per-window slot arrays: slots [0, NT_A*128) half A, rest half B
        slot_idx = np.zeros((NW, NT * 128), np.int64)      # biased table row
        dstrel = np.full((NW, NT * 128), -1.0, np.float32)
        wslot = np.where(is_a, pos, NT_A * 128 + pos)
        slot_idx[w_of, wslot] = np.where(is_a, c_src, c_src - HALF)
        dstrel[w_of, wslot] = (c_dst - base - (w_of << 7)).astype(np.float32)

        ixa = np.zeros((128, TOT_A), np.int16)
        ixb = np.zeros((128, TOT_B), np.int16)
        for gi, (ws, gn) in enumerate(cfg.groups):
            seq_a = slot_idx[ws:ws + gn, :NT_A * 128].reshape(-1)
            seq_b = slot_idx[ws:ws + gn, NT_A * 128:].reshape(-1)
            ixa[:, offA[gi]:offA[gi + 1]] = rep8(wrap16(seq_a))
            ixb[:, offB[gi]:offB[gi + 1]] = rep8(wrap16(seq_b))

        # dstrel as SBUF layout [128, NW*NT]: col w*NT+t, partition p
        drel = np.ascontiguousarray(
            dstrel.reshape(NW * NT, 128).T.astype(np.float32))

        sl = slice(base, base + NWS)
        nd = lambda a: np.ascontiguousarray(a[sl].reshape(NW, 128).T)
        degT = np.broadcast_to(deg_ext[sl][None, :], (128, NWS)).copy()

        in_maps.append({
            "ixa": ixa, "ixb": ixb,
            "ei_dstrel": drel,
            "nd_deg": nd(deg_ext),
            "nd_x": nd(x_ext),
            "nd_batch": nd(batch_ext),
            "degT": degT,
            "iota": iota,
            "cnts": cnts2,
            "w1": np.asarray(W1, np.float32).reshape(1, 64),
            "w2": np.asarray(W2, np.float32).reshape(64, 128),
            "w3": np.asarray(W3, np.float32).reshape(128, 64),
            "b1": np.asarray(b1, np.float32).reshape(64, 1),
            "b2": np.asarray(b2, np.float32).reshape(128, 1),
            "b3": np.asarray(b3, np.float32).reshape(64, 1),
            "l1w": np.asarray(lin1_w, np.float32).reshape(64, 32),
            "l1b": np.asarray(lin1_b, np.float32).reshape(32, 1),
            "l2w": np.asarray(lin2_w, np.float32).reshape(32, 1),
            "l2b": np.full((128, 1),
                           np.float32(np.asarray(lin2_b).reshape(())),
                           np.float32),
        })
    return in_maps, (TOT_A, TOT_B, offA, offB)


# ----------------------------------------------------------------------------
# Device program
# ----------------------------------------------------------------------------

def build_program(cfg, TOT_A, TOT_B, offA, offB):
    NW, NT, NWS, PAD_N, GP = cfg.NW, cfg.NT, cfg.NWS, cfg.PAD_N, cfg.GP
    NT_A, NT_B, HALF = cfg.NT_A, cfg.NT_B, cfg.HALF
    rg = [list(range(cfg.C))]

    nc = bacc.Bacc("TRN2", target_bir_lowering=False, debug=False,
                   num_devices=cfg.C)

    din = {}
    def inp(name, shape, dt=F32):
        din[name] = nc.dram_tensor(name, list(shape), dt, kind="ExternalInput")
        return din[name]

    inp("ixa", (128, TOT_A), I16)
    inp("ixb", (128, TOT_B), I16)
    inp("ei_dstrel", (128, NW * NT))
    inp("nd_deg", (128, NW))
    inp("nd_x", (128, NW))
    inp("nd_batch", (128, NW))
    inp("degT", (128, NWS))
    inp("iota", (128, GP))
    inp("cnts", (128, 2))
    inp("w1", (1, 64)); inp("w2", (64, 128)); inp("w3", (128, 64))
    inp("b1", (64, 1)); inp("b2", (128, 1)); inp("b3", (64, 1))
    inp("l1w", (64, 32)); inp("l1b", (32, 1)); inp("l2w", (32, 1))
    inp("l2b", (128, 1))

    out_d = nc.dram_tensor("out", [cfg.GP, 1], F32, kind="ExternalOutput")

    y_sl = nc.dram_tensor("y_slice", [NWS, 64], F32, kind="Internal")
    y_full = nc.dram_tensor("y_full", [PAD_N, 64], F32, kind="Internal",
                            addr_space="Shared")
    g2_sl = nc.dram_tensor("g2_slice", [NWS, 64], F32, kind="Internal")
    g2_full = nc.dram_tensor("g2_full", [PAD_N, 64], F32, kind="Internal",
                             addr_space="Shared")
    g3_sl = nc.dram_tensor("g3_slice", [NWS, 64], F32, kind="Internal")
    g3_full = nc.dram_tensor("g3_full", [PAD_N, 64], F32, kind="Internal",
                             addr_space="Shared")
    pool_in = nc.dram_tensor("pool_in", [GP, 64], F32, kind="Internal")
    pool_out = nc.dram_tensor("pool_out", [GP, 64], F32, kind="Internal",
                              addr_space="Shared")

    with tile.TileContext(nc) as tc, ExitStack() as ctx:
        P = ctx.enter_context
        setup = P(tc.tile_pool(name="setup", bufs=1))
        oh_pool = P(tc.tile_pool(name="oh", bufs=2))
        gb_pool = P(tc.tile_pool(name="gb", bufs=2))
        fn_pool = P(tc.tile_pool(name="fn", bufs=2))
        psS = P(tc.tile_pool(name="psS", bufs=2, space="PSUM"))
        psZ = P(tc.tile_pool(name="psZ", bufs=2, space="PSUM"))
        psT = P(tc.tile_pool(name="psT", bufs=2, space="PSUM"))
        psHold = P(tc.tile_pool(name="psHold", bufs=1, space="PSUM"))
        ev1 = P(tc.tile_pool(name="ev1", bufs=2))
        ev2 = P(tc.tile_pool(name="ev2", bufs=2))
        ev3 = P(tc.tile_pool(name="ev3", bufs=2))
        stg = P(tc.tile_pool(name="stg", bufs=1))

        def load(name, shape, dt=F32):
            t = setup.tile(list(shape), dt, tag=name)
            nc.sync.dma_start(out=t[:], in_=din[name].ap()[:])
            return t

        ixa = load("ixa", (128, TOT_A), I16)
        ixb = load("ixb", (128, TOT_B), I16)
        dstrel = load("ei_dstrel", (128, NW * NT))
        nd_deg = load("nd_deg", (128, NW))
        nd_x = load("nd_x", (128, NW))
        nd_batch = load("nd_batch", (128, NW))
        degT = load("degT", (128, NWS))
        iota = load("iota", (128, GP))
        cnts = load("cnts", (128, 2))
        w1 = load("w1", (1, 64)); w2 = load("w2", (64, 128))
        w3 = load("w3", (128, 64))
        b1 = load("b1", (64, 1)); b2 = load("b2", (128, 1))
        b3 = load("b3", (64, 1))
        l1w = load("l1w", (64, 32)); l1b = load("l1b", (32, 1))
        l2w = load("l2w", (32, 1)); l2b = load("l2b", (128, 1))

        ident = setup.tile([128, 128], F32, tag="ident")
        make_identity(nc, ident[:])
        ones_col = setup.tile([128, 1], F32, tag="ones")
        nc.vector.memset(ones_col[:], 1.0)
        fence_ix = setup.tile([128, 8], I16, tag="fence_ix")
        nc.vector.memset(fence_ix[:], 0)

        dinvT = setup.tile([128, NWS], F32, tag="dinvT")
        nc.scalar.activation(out=dinvT[:], in_=degT[:], func=AF.Sqrt,
                             bias=1.0, scale=1.0)
        nc.vector.reciprocal(out=dinvT[:], in_=dinvT[:])
        dinv_nm = setup.tile([128, NW], F32, tag="dinv_nm")
        nc.scalar.activation(out=dinv_nm[:], in_=nd_deg[:], func=AF.Sqrt,
                             bias=1.0, scale=1.0)
        nc.vector.reciprocal(out=dinv_nm[:], in_=dinv_nm[:])
        y_nm = setup.tile([128, NW], F32, tag="y_nm")
        nc.vector.tensor_tensor(out=y_nm[:], in0=nd_x[:], in1=dinv_nm[:],
                                op=OP.mult)

        staging = stg.tile([128, NW * 64], F32, tag="staging")

        # y table: y replicated to 64 lanes, node-major
        nc.vector.tensor_copy(
            out=staging[:].rearrange("p (w f) -> p w f", f=64),
            in_=y_nm[:, :, None].to_broadcast([128, NW, 64]))
        nc.sync.dma_start(
            out=y_sl.ap()[:].rearrange("(w p) f -> p w f", p=128),
            in_=staging[:].rearrange("p (w f) -> p w f", f=64))
        nc.gpsimd.collective_compute(
            "AllGather", OP.bypass, replica_groups=rg,
            ins=[y_sl.ap()[:]], outs=[y_full.ap()[:]])

        def gather_group(gi, gn, table):
            """fenced grouped gather; returns (gb_tile, fence_inst)."""
            gb = gb_pool.tile([128, GSZ * NT * 64], F32, tag="gb")
            nA, nB = gn * NT_A * 128, gn * NT_B * 128
            callA = nc.gpsimd.dma_gather(
                out_ap=gb[:, :nA // 128 * 64].rearrange(
                    "p (t f) -> p t f", f=64),
                in_ap=table.ap()[:HALF, :],
                idxs_ap=ixa[:, offA[gi]:offA[gi + 1]],
                num_idxs=nA, num_idxs_reg=nA, elem_size=64,
                single_packet=False)
            callB = nc.gpsimd.dma_gather(
                out_ap=gb[:, nA // 128 * 64:(nA + nB) // 128 * 64].rearrange(
                    "p (t f) -> p t f", f=64),
                in_ap=table.ap()[HALF:, :],
                idxs_ap=ixb[:, offB[gi]:offB[gi + 1]],
                num_idxs=nB, num_idxs_reg=nB, elem_size=64,
                single_packet=False)
            fence_t = fn_pool.tile([128, 64], F32, tag="fence")
            fence = nc.gpsimd.dma_gather(
                out_ap=fence_t[:].rearrange("p (t f) -> p t f", f=64),
                in_ap=table.ap()[:HALF, :],
                idxs_ap=fence_ix[:],
                num_idxs=128, num_idxs_reg=128, elem_size=64,
                single_packet=True)
            add_dep_helper(fence.ins, callA.ins, True, "fence>A")
            add_dep_helper(fence.ins, callB.ins, True, "fence>B")
            return gb, fence

        def onehot_win(w):
            oh = oh_pool.tile([128, NT * 128], F32, tag="oh")
            dr3 = dstrel[:, w * NT:(w + 1) * NT][:, :, None].to_broadcast(
                [128, NT, 128])
            io3 = iota[:, None, :128].to_broadcast([128, NT, 128])
            nc.vector.tensor_tensor(
                out=oh[:].rearrange("p (t j) -> p t j", j=128),
                in0=dr3, in1=io3, op=OP.is_equal)
            return oh

        def scatter_win(oh, gb, fence, wl, gn, F):
            ps = psS.tile([F, 128], F32, space="PSUM", tag="psS")
            for t in range(NT):
                c = chunk_of(cfg, wl, t, gn)
                mm = nc.tensor.matmul(
                    out=ps[:], lhsT=gb[:, c * 64:c * 64 + F],
                    rhs=oh[:, t * 128:(t + 1) * 128],
                    start=(t == 0), stop=(t == NT - 1))
                add_dep_helper(mm.ins, fence.ins, True, "mm>fence")
            return ps

        def wsl(w):
            return slice(w * 128, (w + 1) * 128)

        # ---- Layer 1 --------------------------------------------------------
        for gi, (ws, gn) in enumerate(cfg.groups):
            gb, fence = gather_group(gi, gn, y_full)
            for wl in range(gn):
                w = ws + wl
                oh = onehot_win(w)
                ps1 = scatter_win(oh, gb, fence, wl, gn, 1)
                s1 = ev1.tile([1, 128], F32, tag="s1")
                nc.scalar.activation(out=s1[:], in_=ps1[:], func=AF.Copy)
                psO = psZ.tile([64, 128], F32, space="PSUM", tag="psz")
                nc.tensor.matmul(out=psO[:], lhsT=w1[:], rhs=s1[:],
                                 start=True, stop=True)
                m1 = ev2.tile([64, 128], F32, tag="m1")
                nc.vector.tensor_tensor(out=m1[:], in0=psO[:],
                                        in1=dinvT[:64, wsl(w)], op=OP.mult)
                h1 = ev3.tile([64, 128], F32, tag="h1")
                nc.scalar.activation(out=h1[:], in_=m1[:], func=AF.Relu,
                                     bias=b1[:])
                g2 = ev2.tile([64, 128], F32, tag="g2")
                nc.vector.tensor_tensor(out=g2[:], in0=h1[:],
                                        in1=dinvT[:64, wsl(w)], op=OP.mult)
                psN = psT.tile([128, 64], F32, space="PSUM", tag="psN")
                nc.tensor.transpose(out=psN[:], in_=g2[:],
                                    identity=ident[:64, :64])
                nc.scalar.activation(out=staging[:, w * 64:(w + 1) * 64],
                                     in_=psN[:], func=AF.Copy)

        nc.sync.dma_start(
            out=g2_sl.ap()[:].rearrange("(w p) f -> p w f", p=128),
            in_=staging[:].rearrange("p (w f) -> p w f", f=64))
        nc.gpsimd.collective_compute(
            "AllGather", OP.bypass, replica_groups=rg,
            ins=[g2_sl.ap()[:]], outs=[g2_full.ap()[:]])

        # ---- Layer 2 (+ fold W3, produce g3) --------------------------------
        for gi, (ws, gn) in enumerate(cfg.groups):
            gb, fence = gather_group(gi, gn, g2_full)
            for wl in range(gn):
                w = ws + wl
                oh = onehot_win(w)
                ps2 = scatter_win(oh, gb, fence, wl, gn, 64)
                aggT = ev1.tile([64, 128], F32, tag="aggT")
                nc.vector.tensor_tensor(out=aggT[:], in0=ps2[:],
                                        in1=dinvT[:64, wsl(w)], op=OP.mult)
                psz = psZ.tile([128, 128], F32, space="PSUM", tag="psz")
                nc.tensor.matmul(out=psz[:], lhsT=w2[:], rhs=aggT[:],
                                 start=True, stop=True)
                h2 = ev2.tile([128, 128], F32, tag="h2")
                nc.scalar.activation(out=h2[:], in_=psz[:], func=AF.Relu,
                                     bias=b2[:])
                pst3 = psZ.tile([64, 128], F32, space="PSUM", tag="psz")
                nc.tensor.matmul(out=pst3[:], lhsT=w3[:], rhs=h2[:],
                                 start=True, stop=True)
                g3 = ev3.tile([64, 128], F32, tag="g3")
                nc.vector.tensor_tensor(out=g3[:], in0=pst3[:],
                                        in1=dinvT[:64, wsl(w)], op=OP.mult)
                psN = psT.tile([128, 64], F32, space="PSUM", tag="psN")
                nc.tensor.transpose(out=psN[:], in_=g3[:],
                                    identity=ident[:64, :64])
                nc.scalar.activation(out=staging[:, w * 64:(w + 1) * 64],
                                     in_=psN[:], func=AF.Copy)

        nc.sync.dma_start(
            out=g3_sl.ap()[:].rearrange("(w p) f -> p w f", p=128),
            in_=staging[:].rearrange("p (w f) -> p w f", f=64))
        nc.gpsimd.collective_compute(
            "AllGather", OP.bypass, replica_groups=rg,
            ins=[g3_sl.ap()[:]], outs=[g3_full.ap()[:]])

        # ---- Layer 3 + pooling ----------------------------------------------
        pooled_a = psHold.tile([128, 64], F32, space="PSUM", tag="pool_a")
        pooled_b = psHold.tile([128, 64], F32, space="PSUM", tag="pool_b")
        for gi, (ws, gn) in enumerate(cfg.groups):
            gb, fence = gather_group(gi, gn, g3_full)
            for wl in range(gn):
                w = ws + wl
                oh = onehot_win(w)
                ps3 = scatter_win(oh, gb, fence, wl, gn, 64)
                agg3 = ev1.tile([64, 128], F32, tag="aggT")
                nc.vector.tensor_tensor(out=agg3[:], in0=ps3[:],
                                        in1=dinvT[:64, wsl(w)], op=OP.mult)
                h3 = ev2.tile([64, 128], F32, tag="h3")
                nc.scalar.activation(out=h3[:], in_=agg3[:], func=AF.Relu,
                                     bias=b3[:])
                psN = psT.tile([128, 64], F32, space="PSUM", tag="psN")
                nc.tensor.transpose(out=psN[:], in_=h3[:],
                                    identity=ident[:64, :64])
                h3nm = ev3.tile([128, 64], F32, tag="h3nm")
                nc.scalar.activation(out=h3nm[:], in_=psN[:], func=AF.Copy)
                ohp = oh_pool.tile([128, GP], F32, tag="ohp")
                bc = nd_batch[:, w:w + 1].to_broadcast([128, GP])
                nc.vector.tensor_tensor(out=ohp[:], in0=bc, in1=iota[:],
                                        op=OP.is_equal)
                for half, ps_pool in ((0, pooled_a), (1, pooled_b)):
                    lhs = ohp[:, half * 128:(half + 1) * 128]
                    nc.tensor.matmul(out=ps_pool[:], lhsT=lhs,
                                     rhs=h3nm[:],
                                     start=(w == 0), stop=(w == NW - 1))

        # ---- finale ---------------------------------------------------------
        pa = setup.tile([128, 64], F32, tag="pa")
        pb = setup.tile([128, 64], F32, tag="pb")
        nc.scalar.activation(out=pa[:], in_=pooled_a[:], func=AF.Copy)
        nc.scalar.activation(out=pb[:], in_=pooled_b[:], func=AF.Copy)
        nc.sync.dma_start(out=pool_in.ap()[0:128, :], in_=pa[:])
        nc.sync.dma_start(out=pool_in.ap()[128:256, :], in_=pb[:])
        nc.gpsimd.collective_compute(
            "AllReduce", OP.add, replica_groups=rg,
            ins=[pool_in.ap()[:]], outs=[pool_out.ap()[:]])

        meanT = setup.tile([64, 256], F32, tag="meanT")
        for half in (0, 1):
            pl = setup.tile([128, 64], F32, tag=f"pl{half}")
            nc.sync.dma_start(
                out=pl[:], in_=pool_out.ap()[half * 128:(half + 1) * 128, :])
            cntm = setup.tile([128, 1], F32, tag=f"cntm{half}")
            nc.vector.tensor_scalar_max(out=cntm[:],
                                        in0=cnts[:, half:half + 1],
                                        scalar1=1.0)
            rc = setup.tile([128, 1], F32, tag=f"rc{half}")
            nc.vector.reciprocal(out=rc[:], in_=cntm[:])
            mean = setup.tile([128, 64], F32, tag=f"mean{half}")
            nc.vector.tensor_scalar_mul(out=mean[:], in0=pl[:],
                                        scalar1=rc[:])
            psMT = psT.tile([64, 128], F32, space="PSUM", tag="psN")
            nc.tensor.transpose(out=psMT[:], in_=mean[:], identity=ident[:])
            nc.scalar.activation(out=meanT[:, half * 128:(half + 1) * 128],
                                 in_=psMT[:], func=AF.Copy)

        psZ1 = psZ.tile([32, 256], F32, space="PSUM", tag="psz")
        nc.tensor.matmul(out=psZ1[:], lhsT=l1w[:], rhs=meanT[:],
                         start=True, stop=True)
        z1 = setup.tile([32, 256], F32, tag="z1")
        nc.scalar.activation(out=z1[:], in_=psZ1[:], func=AF.Relu,
                             bias=l1b[:])
        for half in (0, 1):
            psO = psT.tile([128, 1], F32, space="PSUM", tag="psN")
            nc.tensor.matmul(out=psO[:],
                             lhsT=z1[:, half * 128:(half + 1) * 128],
                             rhs=l2w[:], start=True, stop=True)
            ob = setup.tile([128, 1], F32, tag=f"ob{half}")
            nc.scalar.activation(out=ob[:], in_=psO[:], func=AF.Identity,
                                 bias=l2b[:])
            nc.sync.dma_start(out=out_d.ap()[half * 128:(half + 1) * 128, :],
                              in_=ob[:])

    nc.compile()
    return nc


# ----------------------------------------------------------------------------
# Runner
# ----------------------------------------------------------------------------

_CACHE = {}


def get_program(cfg, meta):
    TOT_A, TOT_B, offA, offB = meta
    key = cfg.key()
    if key not in _CACHE:
        _CACHE[key] = build_program(cfg, TOT_A, TOT_B, offA, offB)
    return _CACHE[key]


def run(cfg, inputs, trace=False):
    in_maps, meta = prep_inputs(cfg, **inputs)
    nc = get_program(cfg, meta)
    res = bass_utils.run_bass_kernel_spmd(
        nc, in_maps, core_ids=list(range(cfg.C)), trace=trace)
    out = res.results[0]["out"][:cfg.G, :].astype(np.float32)
    return out, res


def make_cfg(inputs, n_nodes=50000, n_edges=600000, n_graphs=256,
             nw=49, half=32768):
    proto = Cfg(n_nodes, n_edges, n_graphs, 8, nw, 1, 1, half)
    nt_a, nt_b = compute_caps(n_nodes, inputs["edge_index"], proto)
    return Cfg(n_nodes, n_edges, n_graphs, 8, nw, nt_a, nt_b, half)


def kernel(**inputs) -> np.ndarray:
    cfg = make_cfg(inputs)
    out, _ = run(cfg, inputs)
    return out
